# revision 7
# baseline (speedup 1.0000x reference)
"""Trainium2 Bass kernel for nn_AttentionBlock (GroupNorm + single-head self-attention).

Contract: kernel(**inputs) takes FULL unsharded inputs (as produced by
setup_inputs) and returns the FULL [32, 512, 32, 32] float32 output.
Internally shards batch-parallel over 8 NeuronCores (4 batches each).

Algorithm per batch (channels-on-partitions layout, fp32r matmuls):
  1. GroupNorm stats: bn_stats per 128-channel tile, group aggregation via
     small fp32 matmuls with indicator matrices (32 groups x 16 channels).
  2. h = a*x + b (a=rstd*gamma, b=beta-mean*a), rounded to fp32r.
  3. q,k = W_qk^T-style matmuls (weights pre-transposed on PE);
     v^T computed directly as h^T @ W_v^T (swapped-operand matmul).
  4. Scores pass A: s = q^T k; ACT exp with accumulate gives row sums
     (softmax denominators) without keeping E.
  5. Scores pass B: s^T = k^T q with an extra rank-1 matmul subtracting
     sqrt(C)*log(sum) per column -> exp gives normalized E^T directly.
  6. att = (v^T)^T E^T  (c-on-partitions), proj matmul, +x residual, DMA out.
"""
import math

import numpy as np

import concourse.bacc as bacc
import concourse.bass as bass
import concourse.mybir as mybir
import concourse.tile as tile
from concourse import bass_utils
from concourse.masks import make_identity

F32 = mybir.dt.float32
F32R = mybir.dt.float32r
AF = mybir.ActivationFunctionType
OP = mybir.AluOpType

N_CORES = 8
B_FULL, C, H, W = 32, 512, 32, 32
N = H * W  # 1024
BPC = B_FULL // N_CORES  # 4 batches per core
GROUPS = 32
GSIZE = C // GROUPS  # 16 channels per group
EPS = 1e-5
SCALE = 1.0 / math.sqrt(C)  # applied inside exp
CT = C // 128  # 4 channel tiles
NT = N // 128  # 8 pixel tiles

_CACHE = {}


def _build():
    nc = bacc.Bacc("TRN2", target_bir_lowering=False, debug=False)

    x_s = nc.dram_tensor("x_s", [BPC, C, N], F32, kind="ExternalInput").ap()
    qkv_w = nc.dram_tensor("qkv_w", [3 * C, C], F32, kind="ExternalInput").ap()
    qkv_b = nc.dram_tensor("qkv_b", [3 * C], F32, kind="ExternalInput").ap()
    proj_w = nc.dram_tensor("proj_w", [C, C], F32, kind="ExternalInput").ap()
    proj_b = nc.dram_tensor("proj_b", [C], F32, kind="ExternalInput").ap()
    gamma = nc.dram_tensor("gamma", [C], F32, kind="ExternalInput").ap()
    beta = nc.dram_tensor("beta", [C], F32, kind="ExternalInput").ap()
    out_s = nc.dram_tensor("out_s", [BPC, C, N], F32, kind="ExternalOutput").ap()

    with tile.TileContext(nc) as tc:
        with (
            tc.tile_pool(name="wpool", bufs=1) as wpool,
            tc.tile_pool(name="xpool", bufs=2) as xpool,
            tc.tile_pool(name="cpool", bufs=2) as cpool,
            tc.tile_pool(name="qkpool", bufs=1) as qkpool,
            tc.tile_pool(name="vtpool", bufs=1) as vtpool,
            tc.tile_pool(name="etpool", bufs=1) as etpool,
            tc.tile_pool(name="scr", bufs=4) as scr,
            tc.tile_pool(name="small", bufs=2) as small,
            tc.tile_pool(name="rows", bufs=1) as rows,
            tc.tile_pool(name="ps", bufs=8, space="PSUM") as ps,
        ):
            # ---------------- one-time setup ----------------
            with nc.named_scope("setup"):
                ident_f = wpool.tile([128, 128], F32)
                make_identity(nc, ident_f)

                # transposed weights in f32r: wT[c, o] for qkv (o=0..1535), proj.
                # Raw weights staged chunk-wise ([128, 512] per o-tile) via the
                # scratch pool to avoid reserving full-size staging buffers.
                wT_r = wpool.tile([128, CT, 3 * C], F32R)
                pT_r = wpool.tile([128, CT, C], F32R)
                wq_view = qkv_w.rearrange("(ot p) c -> p ot c", p=128)
                wp_view = proj_w.rearrange("(ot p) c -> p ot c", p=128)
                for ot in range(12 + CT):
                    stage = scr.tile([128, 512], F32, tag="scr", name="wstage")
                    if ot < 12:
                        nc.sync.dma_start(out=stage, in_=wq_view[:, ot])
                    else:
                        nc.sync.dma_start(out=stage, in_=wp_view[:, ot - 12])
                    for ct in range(CT):
                        pt = ps.tile([128, 512], F32, tag="mm", name="w_ps")
                        nc.tensor.transpose(
                            pt[:, 0:128], stage[:, bass.ts(ct, 128)], ident_f
                        )
                        s = scr.tile([128, 512], F32, tag="scr", name="wcb")
                        nc.vector.tensor_copy(s[:, 0:128], pt[:, 0:128])
                        if ot < 12:
                            nc.vector.tensor_copy(
                                wT_r[:, ct, bass.ts(ot, 128)], s[:, 0:128]
                            )
                        else:
                            nc.vector.tensor_copy(
                                pT_r[:, ct, bass.ts(ot - 12, 128)], s[:, 0:128]
                            )

                # per-channel params as [128, tiles] columns
                gamma_col = wpool.tile([128, CT], F32)
                nc.sync.dma_start(out=gamma_col, in_=gamma.rearrange("(t p) -> p t", p=128))
                beta_col = wpool.tile([128, CT], F32)
                nc.sync.dma_start(out=beta_col, in_=beta.rearrange("(t p) -> p t", p=128))
                pb_col = wpool.tile([128, CT], F32)
                nc.sync.dma_start(out=pb_col, in_=proj_b.rearrange("(t p) -> p t", p=128))
                qb_col = wpool.tile([128, 2 * CT], F32)
                nc.sync.dma_start(
                    out=qb_col, in_=qkv_b[0 : 2 * C].rearrange("(t p) -> p t", p=128)
                )
                vb_row = wpool.tile([1, C], F32)
                nc.sync.dma_start(out=vb_row, in_=qkv_b[2 * C : 3 * C][None, :])
                vb_bcast = wpool.tile([128, C], F32)
                nc.gpsimd.partition_broadcast(vb_bcast, vb_row)

                eps_col = wpool.tile([128, 1], F32)
                nc.vector.memset(eps_col, EPS)
                ones_f = wpool.tile([1, 128], F32)
                nc.vector.memset(ones_f, 1.0)
                ones_r = wpool.tile([1, 128], F32R)
                nc.vector.tensor_copy(ones_r, ones_f)

                # group indicator matrices (fp32, used in fp32 matmuls)
                # Gt[p, g] = 1/16 if p // 16 == g - 8t  (iota = 128t + p - 16g)
                G_t = wpool.tile([128, CT, GROUPS], F32)
                nc.gpsimd.memset(G_t, 1.0 / GSIZE)
                # St[g, p] = 1 if same condition     (iota = 128t - 16g + p)
                S_t = wpool.tile([GROUPS, CT, 128], F32)
                nc.gpsimd.memset(S_t, 1.0)
                for t in range(CT):
                    for (tile_ap, cm, pat) in (
                        (G_t[:, t], 1, -GSIZE),
                        (S_t[:, t], -GSIZE, 1),
                    ):
                        n_free = tile_ap.shape[-1]
                        nc.gpsimd.affine_select(
                            out=tile_ap, in_=tile_ap,
                            compare_op=OP.is_ge, fill=0.0,
                            base=128 * t, channel_multiplier=cm,
                            pattern=[[pat, n_free]],
                        )
                        # (iota <= GSIZE-1)  ==  (-iota + GSIZE-1 >= 0); is_le
                        # is not implemented in walrus codegen, so negate.
                        nc.gpsimd.affine_select(
                            out=tile_ap, in_=tile_ap,
                            compare_op=OP.is_ge, fill=0.0,
                            base=(GSIZE - 1) - 128 * t, channel_multiplier=-cm,
                            pattern=[[-pat, n_free]],
                        )

            # ---------------- per-batch pipeline ----------------
            for b in range(BPC):
                x_t = xpool.tile([128, CT, N], F32, tag="x", name="x_t")

                with nc.named_scope("load"):
                    nc.sync.dma_start(
                        out=x_t, in_=x_s[b].rearrange("(t p) n -> p t n", p=128)
                    )

                with nc.named_scope("stats"):
                    # per-partition mean/var over the N pixels, then mean^2
                    stats3 = small.tile([128, CT, 3], F32, tag="stats3", name="stats3")
                    for t in range(CT):
                        bnst = small.tile([128, 2, 6], F32, tag="bnst", name="bnst")
                        for s2 in range(2):
                            nc.vector.bn_stats(
                                out=bnst[:, s2], in_=x_t[:, t, bass.ts(s2, 512)]
                            )
                        nc.vector.bn_aggr(out=stats3[:, t, 0:2], in_=bnst)
                        nc.vector.tensor_mul(
                            stats3[:, t, 2:3], stats3[:, t, 0:1], stats3[:, t, 0:1]
                        )
                    # aggregate over groups: [32, 3] = sum_t G_t^T @ stats3_t / 16
                    agg_ps = ps.tile([128, 512], F32, tag="mm", name="agg_ps")
                    for t in range(CT):
                        # stats are means over N pixels; G holds 1/(16*N)... see scale fix below
                        nc.tensor.matmul(
                            agg_ps[0:GROUPS, 0:3], G_t[:, t], stats3[:, t],
                            start=(t == 0), stop=(t == CT - 1),
                        )
                    agg = small.tile([128, 8], F32, tag="agg", name="agg")
                    nc.vector.tensor_copy(agg[0:GROUPS, 0:3], agg_ps[0:GROUPS, 0:3])
                    # mean_g=agg0, E[var]=agg1, E[m^2]=agg2 (all already /16 via G)
                    nc.vector.tensor_mul(agg[0:GROUPS, 3:4], agg[0:GROUPS, 0:1], agg[0:GROUPS, 0:1])
                    nc.vector.tensor_add(agg[0:GROUPS, 4:5], agg[0:GROUPS, 1:2], agg[0:GROUPS, 2:3])
                    nc.vector.tensor_tensor(
                        agg[0:GROUPS, 5:6], agg[0:GROUPS, 4:5], agg[0:GROUPS, 3:4], OP.subtract
                    )
                    nc.scalar.activation(
                        out=agg[0:GROUPS, 6:7], in_=agg[0:GROUPS, 5:6],
                        func=AF.Sqrt, bias=eps_col[0:GROUPS], scale=1.0,
                    )
                    nc.vector.reciprocal(agg[0:GROUPS, 7:8], agg[0:GROUPS, 6:7])
                    mr = small.tile([128, 2], F32, tag="mr", name="mr")
                    nc.vector.tensor_copy(mr[0:GROUPS, 0:1], agg[0:GROUPS, 0:1])
                    nc.vector.tensor_copy(mr[0:GROUPS, 1:2], agg[0:GROUPS, 7:8])
                    # scatter group mean/rstd back to channel partitions
                    mrcol = small.tile([128, CT, 2], F32, tag="mrcol", name="mrcol")
                    for t in range(CT):
                        sc_ps = ps.tile([128, 512], F32, tag="mm", name="sc_ps")
                        nc.tensor.matmul(
                            sc_ps[:, 0:2], S_t[0:GROUPS, t], mr[0:GROUPS],
                            start=True, stop=True,
                        )
                        nc.vector.tensor_copy(mrcol[:, t], sc_ps[:, 0:2])
                    ab = small.tile([128, CT, 2], F32, tag="ab", name="ab")
                    for t in range(CT):
                        nc.vector.tensor_mul(
                            ab[:, t, 0:1], mrcol[:, t, 1:2], gamma_col[:, t : t + 1]
                        )
                        nc.vector.tensor_mul(ab[:, t, 1:2], mrcol[:, t, 0:1], ab[:, t, 0:1])
                        nc.vector.tensor_tensor(
                            ab[:, t, 1:2], beta_col[:, t : t + 1], ab[:, t, 1:2], OP.subtract
                        )

                # h = a*x + b -> fp32r
                h_r = cpool.tile([128, CT, N], F32R, tag="cbuf", name="h_r")
                with nc.named_scope("hnorm"):
                    for t in range(CT):
                        for ch in range(2):
                            s = scr.tile([128, 512], F32, tag="scr", name="hscr")
                            nc.scalar.activation(
                                out=s, in_=x_t[:, t, bass.ts(ch, 512)],
                                func=AF.Identity, bias=ab[:, t, 1:2], scale=ab[:, t, 0:1],
                            )
                            nc.vector.tensor_copy(h_r[:, t, bass.ts(ch, 512)], s)

                # q,k: [128, 8, 1024]  (m-tiles 0..3 = q channels, 4..7 = k)
                qk_r = qkpool.tile([128, 2 * CT, N], F32R, tag="qk", name="qk_r")
                with nc.named_scope("qk"):
                    for m in range(2 * CT):
                        for ch in range(2):
                            p = ps.tile([128, 512], F32, tag="mm", name="qk_ps")
                            for kc in range(CT):
                                nc.tensor.matmul(
                                    p, wT_r[:, kc, bass.ts(m, 128)],
                                    h_r[:, kc, bass.ts(ch, 512)],
                                    start=(kc == 0), stop=(kc == CT - 1),
                                )
                            nc.vector.tensor_tensor(
                                qk_r[:, m, bass.ts(ch, 512)], p,
                                qb_col[:, m : m + 1].to_broadcast([128, 512]), OP.add,
                            )

                # pass A: s = q^T k, exp+accumulate for row sums
                ls_acc = small.tile([128, NT, 2], F32, tag="lsacc", name="ls_acc")
                with nc.named_scope("scoresA"):
                    for m in range(NT):
                        for ch in range(2):
                            p = ps.tile([128, 512], F32, tag="mm", name="sA_ps")
                            for kc in range(CT):
                                nc.tensor.matmul(
                                    p, qk_r[:, kc, bass.ts(m, 128)],
                                    qk_r[:, CT + kc, bass.ts(ch, 512)],
                                    start=(kc == 0), stop=(kc == CT - 1),
                                )
                            s = scr.tile([128, 512], F32, tag="scr", name="eAscr")
                            nc.scalar.activation(
                                out=s, in_=p, func=AF.Exp, bias=0.0, scale=SCALE,
                                accum_out=ls_acc[:, m, ch : ch + 1],
                            )

                # v^T: [128, 8, 512]  (pixels on partitions, channels free)
                vT_r = vtpool.tile([128, NT, C], F32R, tag="vt", name="vT_r")
                with nc.named_scope("vt"):
                    for m in range(NT):
                        p = ps.tile([128, 512], F32, tag="mm", name="vt_ps")
                        for kc in range(CT):
                            nc.tensor.matmul(
                                p, h_r[:, kc, bass.ts(m, 128)],
                                wT_r[:, kc, 2 * C : 3 * C],
                                start=(kc == 0), stop=(kc == CT - 1),
                            )
                        nc.vector.tensor_tensor(vT_r[:, m], p, vb_bcast, OP.add)

                with nc.named_scope("logsum"):
                    ls = small.tile([128, NT], F32, tag="ls", name="ls")
                    nc.vector.tensor_reduce(
                        ls, ls_acc, axis=mybir.AxisListType.X, op=OP.add
                    )
                    lsl = small.tile([128, NT], F32, tag="lsl", name="lsl")
                    nc.scalar.activation(out=lsl, in_=ls, func=AF.Ln)
                    nc.vector.tensor_scalar_mul(lsl, lsl, -1.0 / SCALE)
                    row_f = rows.tile([1, N], F32, tag="rowf", name="row_f")
                    with nc.allow_non_contiguous_dma(
                        reason="4KB cross-partition logsum gather, once per batch"
                    ):
                        for m in range(NT):
                            nc.sync.dma_start(
                                out=row_f[0:1, bass.ts(m, 128)],
                                in_=lsl[:, m : m + 1],
                            )
                    row_r = rows.tile([1, N], F32R, tag="rowr", name="row_r")
                    nc.vector.tensor_copy(row_r, row_f)

                # pass B: s^T with -logsum folded in; exp -> normalized E^T
                ET_r = etpool.tile([128, NT, N], F32R, tag="et", name="ET_r")
                with nc.named_scope("scoresB"):
                    for m in range(NT):
                        for ch in range(2):
                            p = ps.tile([128, 512], F32, tag="mm", name="sB_ps")
                            for kc in range(CT):
                                nc.tensor.matmul(
                                    p, qk_r[:, CT + kc, bass.ts(m, 128)],
                                    qk_r[:, kc, bass.ts(ch, 512)],
                                    start=(kc == 0), stop=False,
                                )
                            nc.tensor.matmul(
                                p, ones_r, row_r[0:1, bass.ts(ch, 512)],
                                start=False, stop=True,
                            )
                            s = scr.tile([128, 512], F32, tag="scr", name="eBscr")
                            nc.scalar.activation(
                                out=s, in_=p, func=AF.Exp, bias=0.0, scale=SCALE
                            )
                            nc.vector.tensor_copy(ET_r[:, m, bass.ts(ch, 512)], s)

                # att[c, i] = sum_j vT[j, c] * ET[j, i]
                att_r = cpool.tile([128, CT, N], F32R, tag="cbuf", name="att_r")
                with nc.named_scope("av"):
                    for m in range(CT):
                        for ch in range(2):
                            p = ps.tile([128, 512], F32, tag="mm", name="av_ps")
                            for j in range(NT):
                                nc.tensor.matmul(
                                    p, vT_r[:, j, bass.ts(m, 128)],
                                    ET_r[:, j, bass.ts(ch, 512)],
                                    start=(j == 0), stop=(j == NT - 1),
                                )
                            nc.vector.tensor_copy(att_r[:, m, bass.ts(ch, 512)], p)

                # out = x + proj_w @ att + proj_b  (written in place into x_t)
                with nc.named_scope("proj"):
                    for m in range(CT):
                        for ch in range(2):
                            p = ps.tile([128, 512], F32, tag="mm", name="pr_ps")
                            for kc in range(CT):
                                nc.tensor.matmul(
                                    p, pT_r[:, kc, bass.ts(m, 128)],
                                    att_r[:, kc, bass.ts(ch, 512)],
                                    start=(kc == 0), stop=(kc == CT - 1),
                                )
                            s = scr.tile([128, 512], F32, tag="scr", name="prscr")
                            nc.vector.tensor_scalar_add(s, p, pb_col[:, m : m + 1])
                            nc.vector.tensor_add(
                                x_t[:, m, bass.ts(ch, 512)], s, x_t[:, m, bass.ts(ch, 512)]
                            )

                with nc.named_scope("store"):
                    nc.sync.dma_start(
                        out=out_s[b].rearrange("(t p) n -> p t n", p=128), in_=x_t
                    )

    nc.compile()
    return nc


def _get_nc():
    if "nc" not in _CACHE:
        _CACHE["nc"] = _build()
    return _CACHE["nc"]


def run(inputs, trace=False):
    nc = _get_nc()
    x = np.ascontiguousarray(np.asarray(inputs["x"], dtype=np.float32)).reshape(
        B_FULL, C, N
    )
    weights = {
        "qkv_w": np.ascontiguousarray(np.asarray(inputs["qkv_w"], np.float32)),
        "qkv_b": np.ascontiguousarray(np.asarray(inputs["qkv_b"], np.float32)),
        "proj_w": np.ascontiguousarray(np.asarray(inputs["proj_w"], np.float32)),
        "proj_b": np.ascontiguousarray(np.asarray(inputs["proj_b"], np.float32)),
        "gamma": np.ascontiguousarray(np.asarray(inputs["norm_gamma"], np.float32)),
        "beta": np.ascontiguousarray(np.asarray(inputs["norm_beta"], np.float32)),
    }
    in_maps = []
    for c in range(N_CORES):
        m = {"x_s": x[c * BPC : (c + 1) * BPC]}
        m.update(weights)
        in_maps.append(m)
    res = bass_utils.run_bass_kernel_spmd(
        nc, in_maps, core_ids=list(range(N_CORES)), trace=trace
    )
    out = np.concatenate([r["out_s"] for r in res.results], axis=0)
    return out.reshape(B_FULL, C, H, W), res


def kernel(**inputs) -> np.ndarray:
    out, _ = run(inputs, trace=False)
    return out
